# revision 7
# baseline (speedup 1.0000x reference)
"""AttentiveManifoldMixer Trainium2 kernel (8-core data parallel over batch).

Math: with W3[c,i,j] = conv_w[c*64+i, j], conv_b = eye(C).flatten(),
  s[b]    = sigmoid(fc2 @ relu(fc1 @ mean_hw(x[b])))
  out[b,c,p] = sum_{i,j} W3[c,i,j] * s[b,j] * x[b,i,p] * x[b,j,p] + x[b,c,p]

The quadratic form is symmetrized over unordered channel pairs grouped by
cyclic diagonal offset d: feature lane (d, i) holds x_i * x_{(i+d)%64}, with
the per-batch weight (W3[c,i,j]*s_j + W3[c,j,i]*s_i)/mult folded on device.
17 chunks x 128 lanes cover d = 0..32 (chunk 16 holds d=32 at mult 4).

Per core: features via one 128-lane bf16 tensor_tensor per chunk against
DMA-built rotated copies of x; GEMM = 17 bf16 matmuls (K=128, M=64, N=512)
accumulating in PSUM + exact fp32 identity-matmul residual.
"""
import sys

sys.path.insert(0, "/opt/trn_rl_repo")

import numpy as np
import ml_dtypes

B, C, H, W = 8, 64, 64, 64
P = H * W                  # 4096 pixels per sample
MID = C // 4
NCHUNK = 17                # feature chunks (d pairs)
PT = 2048                  # pixel tile
NSUB = 512                 # matmul free-dim subtile
N_CORES = 8

_CACHE = {}


def _lane_maps():
    """Per-lane (i, j, mult) for chunk m, lane q (q = 64*qhi + qlo)."""
    i_idx = np.zeros((NCHUNK, 128), np.int64)
    j_idx = np.zeros((NCHUNK, 128), np.int64)
    mult = np.ones((NCHUNK, 128), np.float32)
    for m in range(NCHUNK):
        for q in range(128):
            qhi, qlo = divmod(q, 64)
            d = (2 * m + qhi) if m < 16 else 32
            i_idx[m, q] = qlo
            j_idx[m, q] = (qlo + d) % 64
            if m == 16:
                mult[m, q] = 4.0
    return i_idx, j_idx, mult


def _host_weights(conv_w, fc1_w, fc2_w):
    """Pre-gather conv_w into per-lane arrays a1/a2 of shape (128, 17, 64):
    [lane q, chunk m, out-channel c]."""
    w3 = conv_w.reshape(C, C, C)  # [c, i, j]
    i_idx, j_idx, mult = _lane_maps()
    # a1[q, m, c] = W3[c, i, j] / mult ; a2[q, m, c] = W3[c, j, i] / mult (0 on diag)
    a1 = np.transpose(w3[:, i_idx, j_idx], (2, 1, 0)) / mult.T[:, :, None]
    a2 = np.transpose(w3[:, j_idx, i_idx], (2, 1, 0)) / mult.T[:, :, None]
    diag = (i_idx == j_idx).T  # [q, m]
    a2[diag] = 0.0
    fc1t = (fc1_w.T / float(P)).copy()   # (64, 16): folds the 1/HW of the mean
    fc2t = fc2_w.T.copy()                # (16, 64)
    return (np.ascontiguousarray(a1, np.float32),
            np.ascontiguousarray(a2, np.float32), fc1t, fc2t)


def _build_program():
    import concourse.bacc as bacc
    import concourse.bass as bass
    from concourse import mybir
    from concourse.tile import TileContext

    nc = bacc.Bacc("TRN2", target_bir_lowering=False, debug=False)
    dt = mybir.dt

    x_d = nc.dram_tensor("x", [C, P], dt.float32, kind="ExternalInput")
    a1_d = nc.dram_tensor("a1", [128, NCHUNK, C], dt.float32, kind="ExternalInput")
    a2_d = nc.dram_tensor("a2", [128, NCHUNK, C], dt.float32, kind="ExternalInput")
    f1_d = nc.dram_tensor("fc1t", [C, MID], dt.float32, kind="ExternalInput")
    f2_d = nc.dram_tensor("fc2t", [MID, C], dt.float32, kind="ExternalInput")
    id_d = nc.dram_tensor("ident", [C, C], dt.float32, kind="ExternalInput")
    out_d = nc.dram_tensor("out", [C, P], dt.float32, kind="ExternalOutput")

    NPT = P // PT           # pixel tiles
    NS = PT // NSUB         # psum subtiles per pixel tile

    with TileContext(nc) as tc:
        with tc.tile_pool(name="single", bufs=1) as single, \
             tc.tile_pool(name="dram", bufs=1, space="DRAM") as dpool, \
             tc.tile_pool(name="var", bufs=4) as varp, \
             tc.tile_pool(name="feat", bufs=4) as featp, \
             tc.tile_pool(name="outs", bufs=4) as outsp, \
             tc.tile_pool(name="psum", bufs=6, space="PSUM") as psum, \
             tc.tile_pool(name="psum_se", bufs=2, space="PSUM") as psum_se:

            # ---- load x (fp32, resident) + weights ----
            xf = single.tile([C, P], dt.float32)
            for pt in range(NPT):
                nc.sync.dma_start(out=xf[:, pt * PT:(pt + 1) * PT],
                                  in_=x_d.ap()[:, pt * PT:(pt + 1) * PT])
            a1s = single.tile([128, NCHUNK, C], dt.float32)
            nc.sync.dma_start(out=a1s, in_=a1_d.ap())
            a2s = single.tile([128, NCHUNK, C], dt.float32)
            nc.sync.dma_start(out=a2s, in_=a2_d.ap())
            f1s = single.tile([C, MID], dt.float32)
            nc.sync.dma_start(out=f1s, in_=f1_d.ap())
            f2s = single.tile([MID, C], dt.float32)
            nc.sync.dma_start(out=f2s, in_=f2_d.ap())
            ids = single.tile([C, C], dt.float32)
            nc.sync.dma_start(out=ids, in_=id_d.ap())

            # ---- prestage: cast x -> bf16 (with channel-sum accumulation) ----
            xb = single.tile([C, P], dt.bfloat16)
            sums = single.tile([C, 1], dt.float32)
            nc.scalar.activation(xb, xf, mybir.ActivationFunctionType.Copy,
                                 accum_out=sums)
            xb_dram = dpool.tile([C, P], dt.bfloat16)
            nc.sync.dma_start(out=xb_dram, in_=xb)

            # ---- SE path: s = sigmoid(fc2 @ relu(fc1 @ mean)) ----
            ps1 = psum_se.tile([MID, 1], dt.float32, tag="se")
            nc.tensor.matmul(ps1, f1s, sums, start=True, stop=True)
            y1 = single.tile([MID, 1], dt.float32)
            nc.scalar.activation(y1, ps1, mybir.ActivationFunctionType.Relu)
            ps2 = psum_se.tile([C, 1], dt.float32, tag="se")
            nc.tensor.matmul(ps2, f2s, y1, start=True, stop=True)
            svec = single.tile([C, 1], dt.float32)
            nc.scalar.activation(svec, ps2, mybir.ActivationFunctionType.Sigmoid)

            # s -> DRAM twice (s_int = [s; s]) for the gather DMAs
            s_int = dpool.tile([2 * C], dt.float32)
            nc.sync.dma_start(out=s_int[0:C][:, None], in_=svec)
            nc.sync.dma_start(out=s_int[C:2 * C][:, None], in_=svec)

            # gathers: S1p[q, m] = s[(qlo + 2m + qhi) % 64]  (chunks 0..15)
            s1p = single.tile([128, 16], dt.float32)
            for qhi in range(2):
                nc.sync.dma_start(
                    out=s1p[64 * qhi:64 * qhi + 64, :],
                    in_=bass.AP(tensor=s_int.tensor,
                                offset=s_int.offset + qhi,
                                ap=[[1, 64], [2, 16]]))
            # S1c16[q] = s[(qlo + 32) % 64] ; S2[q] = s[qlo]
            s1c16 = single.tile([128, 1], dt.float32)
            s2v = single.tile([128, 1], dt.float32)
            for qhi in range(2):
                nc.sync.dma_start(
                    out=s1c16[64 * qhi:64 * qhi + 64, :],
                    in_=bass.AP(tensor=s_int.tensor,
                                offset=s_int.offset + 32,
                                ap=[[1, 64], [0, 1]]))
                nc.sync.dma_start(
                    out=s2v[64 * qhi:64 * qhi + 64, :],
                    in_=bass.AP(tensor=s_int.tensor,
                                offset=s_int.offset,
                                ap=[[1, 64], [0, 1]]))

            # ---- fold s into weights: wc = a1*S1 + a2*S2 (bf16) ----
            wc = single.tile([128, NCHUNK, C], dt.bfloat16)
            t1 = single.tile([128, NCHUNK, C], dt.float32)
            t2 = single.tile([128, NCHUNK, C], dt.float32)
            for m in range(NCHUNK):
                s1col = s1p[:, m:m + 1] if m < 16 else s1c16
                nc.scalar.mul(t1[:, m, :], a1s[:, m, :], s1col)
                nc.scalar.mul(t2[:, m, :], a2s[:, m, :], s2v)
                nc.vector.tensor_add(wc[:, m, :], t1[:, m, :], t2[:, m, :])

            # ---- main loop over pixel tiles ----
            for pt in range(NPT):
                px = slice(pt * PT, (pt + 1) * PT)
                x2 = varp.tile([128, PT], dt.bfloat16, tag="x2")
                nc.sync.dma_start(out=x2[0:C, :], in_=xb_dram[:, px])
                nc.sync.dma_start(out=x2[C:128, :], in_=xb_dram[:, px])

                banks = [psum.tile([C, NSUB], dt.float32, tag="acc",
                                   name=f"bank_{pt}_{n}")
                         for n in range(NS)]

                for m in range(NCHUNK):
                    # variant V = [rot_d0(xb); rot_d1(xb)] built by DMA
                    d0, d1 = (2 * m, 2 * m + 1) if m < 16 else (32, 32)
                    v = varp.tile([128, PT], dt.bfloat16, tag="v")
                    for half, d in ((0, d0), (1, d1)):
                        base = 64 * half
                        if d == 0:
                            nc.sync.dma_start(out=v[base:base + C, :],
                                              in_=xb_dram[:, px])
                        else:
                            nc.sync.dma_start(out=v[base:base + C - d, :],
                                              in_=xb_dram[d:C, px])
                            nc.sync.dma_start(out=v[base + C - d:base + C, :],
                                              in_=xb_dram[0:d, px])
                    f = featp.tile([128, PT], dt.bfloat16, tag="f")
                    nc.vector.tensor_mul(f, x2, v)
                    for n in range(NS):
                        nc.tensor.matmul(banks[n], wc[:, m, :],
                                         f[:, n * NSUB:(n + 1) * NSUB],
                                         start=(m == 0), stop=False)

                for n in range(NS):
                    # residual: += I @ x (exact fp32)
                    nc.tensor.matmul(
                        banks[n], ids,
                        xf[:, pt * PT + n * NSUB: pt * PT + (n + 1) * NSUB],
                        start=False, stop=True)
                    ot = outsp.tile([C, NSUB], dt.float32, tag="o")
                    nc.scalar.copy(ot, banks[n])
                    nc.sync.dma_start(
                        out=out_d.ap()[:, pt * PT + n * NSUB:
                                       pt * PT + (n + 1) * NSUB],
                        in_=ot)

    nc.compile()
    return nc


def _get_program():
    if "nc" not in _CACHE:
        _CACHE["nc"] = _build_program()
    return _CACHE["nc"]


def kernel(x, fc1_w, fc2_w, conv_w, conv_b):
    from concourse.bass_utils import run_bass_kernel_spmd

    x = np.asarray(x, np.float32)
    a1, a2, fc1t, fc2t = _host_weights(
        np.asarray(conv_w, np.float32), np.asarray(fc1_w, np.float32),
        np.asarray(fc2_w, np.float32))
    # conv_b contributes sum_i B[c,i]*x_i with B = conv_b.reshape(C, C); the
    # "residual" matmul realizes it with lhsT = B.T (identity-init -> +x).
    ident = np.ascontiguousarray(
        np.asarray(conv_b, np.float32).reshape(C, C).T)
    nc = _get_program()
    in_maps = []
    for b in range(N_CORES):
        in_maps.append({
            "x": np.ascontiguousarray(x[b].reshape(C, P)),
            "a1": a1, "a2": a2, "fc1t": fc1t, "fc2t": fc2t, "ident": ident,
        })
    res = run_bass_kernel_spmd(nc, in_maps, core_ids=list(range(N_CORES)))
    out = np.stack([res.results[b]["out"].reshape(C, H, W)
                    for b in range(N_CORES)], axis=0)
    return out.astype(np.float32)
